# revision 14
# baseline (speedup 1.0000x reference)
"""Trainium2 Bass kernel: MemoryBank EMA scatter update (8-core SPMD).

Contract: kernel(**inputs) takes FULL unsharded numpy inputs, returns FULL
[1, 128, 4096] float32 output. Internally shards the token dim T=8192 across
8 NeuronCores, computes per-shard importance, AllGathers the [T] importance
vector, selects the global top-2048 via a 3-round 32-ary histogram threshold
(exact to ~2e-3 bucket width; empirically selects the exact top-K set),
accumulates per-slot sums via PE matmul, then ReduceScatters [N,D] sums +
counts and applies the EMA write to each core's 16-slot slice.

Throughput design (the bench differences R reps; the HBM stream of h at
~47us/core/rep is the pacing resource):
  - h is DMA'd once as bf16 via SWDGE cast-in-flight (gpsimd dma): no f32
    staging tile and no engine cast pass. sum(h^2) from bf16 is fine: the
    per-element rounding errors cancel in the 4096-term sum (~1e-4 rel).
  - Phase A: ACT does Square+accum (ss), DVE does score (h_bf @ W in bf16,
    scalar_tensor_tensor accum) + membership onehot counts (fused
    is_equal+add stt, 4 instrs/tile).
  - Top-K threshold: x = imp - globalmax + 64 maps all histogram edges to
    compile-time immediates; each edge is ONE accum instruction over the
    gathered [128, 64] importance block (20 edges on DVE is_gt-count, 12 on
    ACT Sign-sum); per-partition partial counts are summed across partitions
    by gpsimd.partition_all_reduce, which leaves the bucket choice identical
    on every partition - no cross-partition broadcasts at all. Each round
    subtracts its bucket base from x, so the final mask is just x > 0.
  - EMA tail runs in a [128, 512] slot-major layout (s,c)->partition so ops
    use all 128 lanes; DMA's AP linearization does the [16,4096]<->[128,512]
    reshape for free.
  - Collectives are software-pipelined across reps: rep r's ReduceScatter is
    issued right after rep r+1's AllGather; rep r's EMA is emitted after rep
    r+1's drains so no engine queue head-of-line blocks on a collective.
  - DMA rings: h stream on SWDGE (it must cast); bounce/tail DMAs on the ACT
    HWDGE ring; input loads on the SP ring.
"""

import sys

sys.path.insert(0, "/opt/trn_rl_repo")

import numpy as np

# ---- problem constants (hardcoded per contract) ----
T = 8192          # tokens
D = 4096          # hidden dim
N_SLOTS = 128
K_RET = 4
TOPK = 2048
EMA_ALPHA = 0.1
M_CORES = 8
TS = T // M_CORES          # 1024 tokens per core
KT = TS // 128             # 8 token tiles per core (local token l = 128*k + p)
NS = N_SLOTS // M_CORES    # 16 slots per core after ReduceScatter
RSW = D + 16               # 4112: sums 0..4095, counts col 4096, zero pad

NB = 32                    # histogram edges per round
NEDVE = 20                 # edges handled by DVE (is_gt count)
NEACT = NB - NEDVE         # edges handled by ACT (Sign sum)
NROUNDS = 3
XRANGE = 64.0              # x = imp - globalmax + XRANGE  =>  x in (0, 64]
STEPS = [XRANGE / NB ** (j + 1) * NB ** 0 for j in range(NROUNDS)]
STEPS = [XRANGE / NB, XRANGE / NB ** 2, XRANGE / NB ** 3]

_CACHE = {}
import os
_NOCC = os.environ.get("KVAR_NOCC", "0") == "1"  # attribution: stub collectives


def _build(reps=1):
    """Build the SPMD Bass program. reps>1 repeats the whole pipeline for
    tunnel-noise-cancelling benchmarks ((T(R)-T(1))/(R-1) = per-rep time)."""
    from concourse import bass, bacc, tile, mybir, bass_isa

    f32 = mybir.dt.float32
    bf16 = mybir.dt.bfloat16
    i32 = mybir.dt.int32
    AF = mybir.ActivationFunctionType
    OP = mybir.AluOpType

    nc = bacc.Bacc("TRN2", target_bir_lowering=False, debug=False,
                   num_devices=M_CORES)

    h_d = nc.dram_tensor("h", [TS, D], f32, kind="ExternalInput")
    attn_d = nc.dram_tensor("attn", [TS, K_RET], f32, kind="ExternalInput")
    si_d = nc.dram_tensor("si", [TS, K_RET], i32, kind="ExternalInput")
    mem_d = nc.dram_tensor("memslice", [NS, D], f32, kind="ExternalInput")
    w_d = nc.dram_tensor("wimp", [1, D], f32, kind="ExternalInput")
    b_d = nc.dram_tensor("bimp", [1, 1], f32, kind="ExternalInput")
    out_d = nc.dram_tensor("out", [NS, D], f32, kind="ExternalOutput")

    groups = [list(range(M_CORES))]

    with tile.TileContext(nc) as tc:
        with (
            tc.tile_pool(name="dram", bufs=1, space="DRAM") as dram,
            tc.tile_pool(name="const", bufs=1) as const,
            tc.tile_pool(name="hbf", bufs=1) as hbf_pool,
            tc.tile_pool(name="work", bufs=1) as work,
            tc.tile_pool(name="psA", bufs=4, space=bass.MemorySpace.PSUM) as psA,
            tc.tile_pool(name="psC", bufs=1, space=bass.MemorySpace.PSUM) as psC,
        ):
            # ---------- constants ----------
            w_bf = const.tile([128, D], bf16, name="w_bf")
            b_pp = const.tile([128, 1], f32, name="b_pp")
            iota_f = const.tile([128, N_SLOTS], f32, name="iota_f")
            ones_bf = const.tile([128, 1], bf16, name="ones_bf")
            zero_pp = const.tile([128, 1], f32, name="zero_pp")
            eps_pp = const.tile([128, 1], f32, name="eps_pp")
            mem_512 = const.tile([128, D // 8], f32, name="mem_512")
            ones16_8 = const.tile([NS, 8], f32, name="ones16_8")
            # ACT Sign edge biases: actb[rnd][:, i] = -(NEDVE+i)*step_rnd
            actb = [const.tile([128, NEACT], f32, name=f"actb{r}")
                    for r in range(NROUNDS)]

            with tc.tile_pool(name="init", bufs=1) as initp:
                iota_i = initp.tile([128, N_SLOTS], i32, name="iota_i")
                # SWDGE DMA casts f32 -> bf16 in flight
                nc.gpsimd.dma_start(out=w_bf[0:1, :], in_=w_d[:])
                nc.gpsimd.partition_broadcast(w_bf[:], w_bf[0:1, :])
                nc.sync.dma_start(out=b_pp[0:1, :], in_=b_d[:])
                nc.gpsimd.partition_broadcast(b_pp[:], b_pp[0:1, :])
                nc.gpsimd.iota(iota_i[:], pattern=[[1, N_SLOTS]], base=0,
                               channel_multiplier=0)
                nc.vector.tensor_copy(iota_f[:], iota_i[:])
                nc.vector.memset(ones_bf[:], 1.0)
                nc.vector.memset(zero_pp[:], 0.0)
                nc.vector.memset(eps_pp[:], 1e-8)
                nc.vector.memset(ones16_8[:], 1.0)
                for r in range(NROUNDS):
                    nc.vector.tensor_scalar(
                        out=actb[r][:], in0=iota_f[:, NEDVE:NB],
                        scalar1=-STEPS[r], scalar2=None, op0=OP.mult)
                # [16,4096] row-major == [128,512] with p = s*8+c (DMA
                # linearizes both APs elementwise)
                nc.sync.dma_start(out=mem_512[:], in_=mem_d[:])

            h_view = h_d.ap().rearrange("(k p) d -> k p d", p=128)
            attn_v = attn_d.ap().rearrange("(k p) j -> p k j", p=128)
            si_v = si_d.ap().rearrange("(k p) j -> p k j", p=128)

            ctx = dict(nc=nc, tc=tc, bass=bass, mybir=mybir, AF=AF, OP=OP,
                       bass_isa=bass_isa, f32=f32, bf16=bf16, i32=i32,
                       dram=dram, work=work, hbf_pool=hbf_pool, psA=psA,
                       psC=psC, groups=groups, h_view=h_view, attn_v=attn_v,
                       si_v=si_v, w_bf=w_bf, b_pp=b_pp, iota_f=iota_f,
                       ones_bf=ones_bf, zero_pp=zero_pp, eps_pp=eps_pp,
                       mem_512=mem_512, ones16_8=ones16_8, actb=actb,
                       out_d=out_d)

            prev = None
            for rep in range(reps):
                prev = _rep_body(ctx, rep, prev)
            # epilogue: last rep's ReduceScatter + EMA
            _emit_rs(ctx, prev)
            _emit_ema(ctx, prev)

    nc.compile()
    return nc


def _emit_rs(ctx, st):
    nc, OP = ctx["nc"], ctx["OP"]
    if _NOCC:
        nc.scalar.dma_start(out=st["rs_out"][:], in_=st["rs_in"][0:NS, :])
    else:
        nc.gpsimd.collective_compute(
            "ReduceScatter", OP.add, replica_groups=ctx["groups"],
            ins=[st["rs_in"].opt()], outs=[st["rs_out"].opt()])


def _emit_ema(ctx, st):
    """EMA write for this core's 16 slots, in [128, 512] slot-major layout
    ((s,c) -> partition s*8+c) so every op uses all 128 lanes."""
    nc, OP, f32, bf16 = ctx["nc"], ctx["OP"], ctx["f32"], ctx["bf16"]
    work, mem_512, out_d = ctx["work"], ctx["mem_512"], ctx["out_d"]
    ones16_8 = ctx["ones16_8"]
    W8 = D // 8

    rs_sums = work.tile([128, W8], bf16, name="rs_sums", tag="rs_sums",
                        bufs=1)
    cnt16 = work.tile([NS, 1], bf16, name="cnt16", tag="cnt16", bufs=2)
    cntc = work.tile([NS, 1], f32, name="cntc", tag="cntc", bufs=2)
    inv = work.tile([NS, 1], f32, name="inv", tag="inv", bufs=2)
    fac = work.tile([NS, 1], f32, name="fac", tag="fac", bufs=2)
    a_sc = work.tile([NS, 1], f32, name="a_sc", tag="a_sc", bufs=2)
    fac1m = work.tile([NS, 1], f32, name="fac1m", tag="fac1m", bufs=2)
    a8 = work.tile([NS, 8], f32, name="a8", tag="a8", bufs=2)
    f8 = work.tile([NS, 8], f32, name="f8", tag="f8", bufs=2)
    a_pp = work.tile([128, 1], f32, name="a_pp", tag="a_pp", bufs=2)
    f_pp = work.tile([128, 1], f32, name="f_pp", tag="f_pp", bufs=2)
    agg = work.tile([128, W8], f32, name="agg", tag="agg", bufs=1)
    out_512 = work.tile([128, W8], f32, name="out_512", tag="out_512",
                        bufs=1)

    nc.scalar.dma_start(out=rs_sums[:], in_=st["rs_out"][:, 0:D])
    nc.scalar.dma_start(out=cnt16[:], in_=st["rs_out"][:, D:D + 1])
    nc.vector.tensor_scalar_max(cntc[:], cnt16[:], 1.0)
    nc.vector.reciprocal(inv[:], cntc[:])
    nc.vector.tensor_scalar(out=fac[:], in0=cnt16[:], scalar1=0.0,
                            scalar2=EMA_ALPHA, op0=OP.is_gt, op1=OP.mult)
    nc.vector.tensor_tensor(out=a_sc[:], in0=fac[:], in1=inv[:], op=OP.mult)
    nc.vector.tensor_scalar(out=fac1m[:], in0=fac[:], scalar1=-1.0,
                            scalar2=1.0, op0=OP.mult, op1=OP.add)
    nc.vector.tensor_scalar(out=a8[:], in0=ones16_8[:],
                            scalar1=a_sc[:, 0:1], scalar2=None, op0=OP.mult)
    nc.vector.tensor_scalar(out=f8[:], in0=ones16_8[:],
                            scalar1=fac1m[:, 0:1], scalar2=None,
                            op0=OP.mult)
    nc.scalar.dma_start(out=a_pp[:], in_=a8[:])
    nc.scalar.dma_start(out=f_pp[:], in_=f8[:])
    nc.vector.tensor_scalar(out=agg[:], in0=mem_512[:],
                            scalar1=f_pp[:, 0:1], scalar2=None, op0=OP.mult)
    nc.vector.scalar_tensor_tensor(
        out=out_512[:], in0=rs_sums[:], scalar=a_pp[:, 0:1], in1=agg[:],
        op0=OP.mult, op1=OP.add)
    nc.scalar.dma_start(out=out_d[:], in_=out_512[:])


def _rep_body(ctx, rep, prev):
    nc, tc, bass = ctx["nc"], ctx["tc"], ctx["bass"]
    mybir, AF, OP = ctx["mybir"], ctx["AF"], ctx["OP"]
    bass_isa = ctx["bass_isa"]
    f32, bf16, i32 = ctx["f32"], ctx["bf16"], ctx["i32"]
    dram, work, hbf_pool = ctx["dram"], ctx["work"], ctx["hbf_pool"]
    psA, psC = ctx["psA"], ctx["psC"]
    h_view, attn_v, si_v = ctx["h_view"], ctx["attn_v"], ctx["si_v"]
    w_bf, b_pp, iota_f = ctx["w_bf"], ctx["b_pp"], ctx["iota_f"]
    ones_bf = ctx["ones_bf"]
    zero_pp, eps_pp = ctx["zero_pp"], ctx["eps_pp"]

    # ---------- DRAM bounce buffers (fresh per rep: no cross-rep WAR) ----
    ag_in = dram.tile([KT, 128], f32, name=f"ag_in{rep}")
    ag_out = dram.tile([1, T], f32, name=f"ag_out{rep}")
    rs_in = dram.tile([N_SLOTS, RSW], bf16, name=f"rs_in{rep}")
    rs_out = dram.tile([NS, RSW], bf16, name=f"rs_out{rep}")

    # ---------- per-token inputs ----------
    attn_sb = work.tile([128, KT, K_RET], f32, name="attn_sb",
                        tag="attn_sb", bufs=2)
    si_sb = work.tile([128, KT, K_RET], i32, name="si_sb", tag="si_sb",
                      bufs=2)
    si_f = work.tile([128, KT, K_RET], f32, name="si_f", tag="si_f", bufs=2)
    nc.sync.dma_start(out=attn_sb[:], in_=attn_v)
    nc.sync.dma_start(out=si_sb[:], in_=si_v)
    nc.vector.tensor_copy(si_f[:], si_sb[:])

    # ---------- per-token stats ----------
    ss = work.tile([128, KT], f32, name="ss", tag="ss", bufs=2)
    score = work.tile([128, KT], f32, name="score", tag="score", bufs=2)
    imp = work.tile([128, KT], f32, name="imp", tag="imp", bufs=2)
    x_loc = work.tile([128, KT], f32, name="x_loc", tag="x_loc", bufs=2)
    mask = work.tile([128, KT], f32, name="mask", tag="mask", bufs=2)

    scr_sq = work.tile([128, D], bf16, name="scr_sq", tag="scr_sq", bufs=1)
    scr_sc = work.tile([128, D], bf16, name="scr_sc", tag="scr_sc", bufs=1)

    h_bf = [hbf_pool.tile([128, D], bf16, name=f"h_bf{k}", tag="h_bf",
                          bufs=10) for k in range(KT)]
    memb0 = [work.tile([128, N_SLOTS], f32, name=f"memb0_{k}",
                       tag="memb0", bufs=16) for k in range(KT)]
    memb = [work.tile([128, N_SLOTS], bf16, name=f"memb{k}", tag="memb",
                      bufs=16) for k in range(KT)]

    # ---------- phase A: SWDGE-cast h stream; ACT ss, DVE score+memb0 ----
    for k in range(KT):
        nc.gpsimd.dma_start(out=h_bf[k][:], in_=h_view[k])
        nc.scalar.activation(scr_sq[:], h_bf[k][:], AF.Square,
                             bias=zero_pp[:, 0:1], accum_out=ss[:, k:k + 1])
        nc.vector.scalar_tensor_tensor(
            out=scr_sc[:], in0=h_bf[k][:], scalar=1.0, in1=w_bf[:],
            op0=OP.mult, op1=OP.mult, accum_out=score[:, k:k + 1])
        # memb0[k] = sum_j onehot(si[:,k,j]) via fused is_equal+add
        nc.vector.tensor_scalar(out=memb0[k][:], in0=iota_f[:],
                                scalar1=si_f[:, k, 0:1], scalar2=None,
                                op0=OP.is_equal)
        for j in range(1, K_RET):
            nc.vector.scalar_tensor_tensor(
                out=memb0[k][:], in0=iota_f[:], scalar=si_f[:, k, j:j + 1],
                in1=memb0[k][:], op0=OP.is_equal, op1=OP.add)

    # ---------- importance ----------
    alog = work.tile([128, KT, K_RET], f32, name="alog", tag="alog", bufs=2)
    ent = work.tile([128, KT], f32, name="ent", tag="ent", bufs=2)
    mag = work.tile([128, KT], f32, name="mag", tag="mag", bufs=2)
    sig = work.tile([128, KT], f32, name="sig", tag="sig", bufs=2)

    nc.scalar.activation(alog[:], attn_sb[:], AF.Ln, bias=eps_pp[:, 0:1])
    nc.vector.tensor_tensor(out=alog[:], in0=attn_sb[:], in1=alog[:],
                            op=OP.mult)
    nc.vector.tensor_reduce(out=ent[:], in_=alog[:],
                            axis=mybir.AxisListType.X, op=OP.add,
                            negate=True)
    nc.scalar.activation(mag[:], ss[:], AF.Sqrt, bias=zero_pp[:, 0:1])
    nc.vector.tensor_scalar(out=ent[:], in0=ent[:],
                            scalar1=1.0 / float(np.log(4.0)), scalar2=1.0,
                            op0=OP.mult, op1=OP.add)
    nc.vector.tensor_tensor(out=imp[:], in0=mag[:], in1=ent[:], op=OP.mult)
    nc.scalar.activation(sig[:], score[:], AF.Sigmoid, bias=b_pp[:, 0:1])
    nc.vector.tensor_tensor(out=imp[:], in0=imp[:], in1=sig[:], op=OP.add)

    # ---------- AllGather importance ----------
    nc.scalar.dma_start(out=ag_in[:].rearrange("a b -> b a"), in_=imp[:])
    if _NOCC:
        for r in range(M_CORES):
            nc.scalar.dma_start(
                out=ag_out[0:1, TS * r:TS * (r + 1)],
                in_=ag_in[:].rearrange("a b -> (a b)").unsqueeze(0))
    else:
        nc.gpsimd.collective_compute(
            "AllGather", OP.bypass, replica_groups=ctx["groups"],
            ins=[ag_in.opt()], outs=[ag_out.opt()])

    # previous rep's ReduceScatter right after this rep's AllGather so the
    # collective queue interleaves reps (throughput pipelining).
    if prev is not None:
        _emit_rs(ctx, prev)

    # ---------- threshold: 3-round 32-ary histogram, immediate edges ----
    GC = T // 128            # 64 gathered-importance columns per partition
    imp_g = work.tile([128, GC], f32, name="imp_g", tag="imp_g", bufs=2)
    xg = work.tile([128, GC], f32, name="xg", tag="xg", bufs=2)
    rmax = work.tile([128, 1], f32, name="rmax", tag="rmax", bufs=2)
    rmax_ar = work.tile([128, 1], f32, name="rmax_ar", tag="rmax_ar",
                        bufs=2)
    scrM = work.tile([128, GC], f32, name="scrM", tag="scrM", bufs=1)
    scrA = work.tile([128, GC], f32, name="scrA", tag="scrA", bufs=1)

    nc.scalar.dma_start(
        out=imp_g[:], in_=ag_out[:].rearrange("o (a b) -> (o a) b", a=128))
    nc.vector.tensor_reduce(out=rmax[:], in_=imp_g[:],
                            axis=mybir.AxisListType.X, op=OP.max)
    nc.gpsimd.partition_all_reduce(rmax_ar[:], rmax[:], channels=128,
                                   reduce_op=bass_isa.ReduceOp.max)
    # x = imp - max + XRANGE  (same instruction for gathered + local views)
    nc.vector.tensor_scalar(out=xg[:], in0=imp_g[:],
                            scalar1=rmax_ar[:, 0:1], scalar2=XRANGE,
                            op0=OP.subtract, op1=OP.add)
    nc.vector.tensor_scalar(out=x_loc[:], in0=imp[:],
                            scalar1=rmax_ar[:, 0:1], scalar2=XRANGE,
                            op0=OP.subtract, op1=OP.add)

    for rnd in range(NROUNDS):
        step = STEPS[rnd]
        Cd = work.tile([128, NEDVE], f32, name="Cd", tag="Cd", bufs=2)
        Ca = work.tile([128, NEACT], f32, name="Ca", tag="Ca", bufs=2)
        Cd_ar = work.tile([128, NEDVE], f32, name="Cd_ar", tag="Cd_ar",
                          bufs=2)
        Ca_ar = work.tile([128, NEACT], f32, name="Ca_ar", tag="Ca_ar",
                          bufs=2)
        for i in range(NEDVE):
            # accum form needs an explicit no-op op1 (+0) to satisfy the
            # 2-op TensorScalarPtrReduce encoding
            nc.vector.tensor_scalar(out=scrM[:], in0=xg[:],
                                    scalar1=float(i) * step, scalar2=0.0,
                                    op0=OP.is_gt, op1=OP.add,
                                    accum_out=Cd[:, i:i + 1])
        for i in range(NEACT):
            nc.scalar.activation(scrA[:], xg[:], AF.Sign,
                                 bias=ctx["actb"][rnd][:, i:i + 1],
                                 accum_out=Ca[:, i:i + 1])
        nc.gpsimd.partition_all_reduce(Cd_ar[:], Cd[:], channels=128,
                                       reduce_op=bass_isa.ReduceOp.add)
        nc.gpsimd.partition_all_reduce(Ca_ar[:], Ca[:], channels=128,
                                       reduce_op=bass_isa.ReduceOp.add)
        # sel: count >= TOPK.  DVE cols hold counts C; ACT cols hold
        # S = sum sign = L - G, and C = (T - S)/2 >= K  <=>  S <= T - 2K.
        selD = work.tile([128, NEDVE], f32, name="selD", tag="selD", bufs=2)
        selA = work.tile([128, NEACT], f32, name="selA", tag="selA", bufs=2)
        sD = work.tile([128, 1], f32, name="sD", tag="sD", bufs=2)
        sA = work.tile([128, 1], f32, name="sA", tag="sA", bufs=2)
        lo = work.tile([128, 1], f32, name="lo", tag="lo", bufs=2)
        nc.vector.tensor_scalar(out=selD[:], in0=Cd_ar[:],
                                scalar1=TOPK - 0.5, scalar2=None,
                                op0=OP.is_gt)
        nc.vector.tensor_scalar(out=selA[:], in0=Ca_ar[:],
                                scalar1=float(T - 2 * TOPK) + 0.5,
                                scalar2=None, op0=OP.is_lt)
        nc.vector.tensor_reduce(out=sD[:], in_=selD[:],
                                axis=mybir.AxisListType.X, op=OP.add)
        nc.vector.tensor_reduce(out=sA[:], in_=selA[:],
                                axis=mybir.AxisListType.X, op=OP.add)
        # lo = (sD + sA - 1) * step
        nc.vector.tensor_tensor(out=lo[:], in0=sD[:], in1=sA[:], op=OP.add)
        nc.vector.tensor_scalar(out=lo[:], in0=lo[:], scalar1=step,
                                scalar2=-step, op0=OP.mult, op1=OP.add)
        # x -= lo  (gathered view for next round; local view for the mask)
        if rnd < NROUNDS - 1:
            xg2 = work.tile([128, GC], f32, name="xg2", tag="xg", bufs=2)
            nc.vector.tensor_scalar(out=xg2[:], in0=xg[:],
                                    scalar1=lo[:, 0:1], scalar2=None,
                                    op0=OP.subtract)
            xg = xg2
        nc.vector.tensor_scalar(out=x_loc[:], in0=x_loc[:],
                                scalar1=lo[:, 0:1], scalar2=None,
                                op0=OP.subtract)

    # ---------- mask + membership ----------
    nc.vector.tensor_scalar(out=mask[:], in0=x_loc[:], scalar1=0.0,
                            scalar2=None, op0=OP.is_gt)
    for k in range(KT):
        nc.vector.tensor_scalar(out=memb[k][:], in0=memb0[k][:],
                                scalar1=1.0, scalar2=mask[:, k:k + 1],
                                op0=OP.min, op1=OP.mult)

    # ---------- membership matmul (2 phases x 4 PSUM banks) ----------
    cnt_ps = psC.tile([128, 1], f32, name="cnt_ps", tag="cnt_ps")
    DCH = 512
    nph = 4
    for phase in range(2):
        d_lo = phase * nph
        ps = [psA.tile([128, DCH], f32, name=f"ps{phase}_{d}", tag="ps")
              for d in range(nph)]
        for k in range(KT):
            st, sp = (k == 0), (k == KT - 1)
            for d in range(nph):
                c0 = (d_lo + d) * DCH
                nc.tensor.matmul(ps[d][:], memb[k][:],
                                 h_bf[k][:, c0:c0 + DCH], start=st, stop=sp)
            if phase == 0:
                nc.tensor.matmul(cnt_ps[:], memb[k][:], ones_bf[:],
                                 start=st, stop=sp)
        for d in range(nph):
            c0 = (d_lo + d) * DCH
            sums_sb = work.tile([128, DCH], bf16, name="sums_sb",
                                tag="sums_sb", bufs=4)
            if d % 2 == 0:
                nc.vector.tensor_copy(sums_sb[:], ps[d][:])
            else:
                nc.scalar.copy(sums_sb[:], ps[d][:])
            nc.scalar.dma_start(out=rs_in[:, c0:c0 + DCH], in_=sums_sb[:])
        if phase == 0:
            cntw = work.tile([128, RSW - D], bf16, name="cntw", tag="cntw",
                             bufs=2)
            nc.vector.memset(cntw[:], 0.0)
            nc.vector.tensor_copy(cntw[:, 0:1], cnt_ps[:])
            nc.scalar.dma_start(out=rs_in[:, D:RSW], in_=cntw[:])

    # previous rep's EMA tail: emitted after this rep's drains so its
    # ReduceScatter has long completed (no engine-queue stall).
    if prev is not None:
        _emit_ema(ctx, prev)

    return dict(rs_in=rs_in, rs_out=rs_out)


def _get_nc():
    if "nc" not in _CACHE:
        _CACHE["nc"] = _build()
    return _CACHE["nc"]


def _make_in_maps(hidden_states, attention_weights, slot_indices, memory,
                  W_imp, b_imp):
    h = np.ascontiguousarray(np.asarray(hidden_states, dtype=np.float32))
    attn = np.ascontiguousarray(np.asarray(attention_weights,
                                           dtype=np.float32))
    si = np.ascontiguousarray(np.asarray(slot_indices).astype(np.int32))
    mem = np.asarray(memory, dtype=np.float32)[0]
    w = np.ascontiguousarray(np.asarray(W_imp, dtype=np.float32)
                             .reshape(1, D))
    b = np.ascontiguousarray(np.asarray(b_imp, dtype=np.float32)
                             .reshape(1, 1))
    in_maps = []
    for i in range(M_CORES):
        t0 = i * TS
        in_maps.append({
            "h": h[t0:t0 + TS],
            "attn": attn[t0:t0 + TS],
            "si": si[t0:t0 + TS],
            "memslice": np.ascontiguousarray(mem[i * NS:(i + 1) * NS]),
            "wimp": w,
            "bimp": b,
        })
    return in_maps


def kernel(hidden_states, attention_weights, slot_indices, memory, W_imp,
           b_imp):
    from concourse.bass_utils import run_bass_kernel_spmd

    nc = _get_nc()
    in_maps = _make_in_maps(hidden_states, attention_weights, slot_indices,
                            memory, W_imp, b_imp)
    res = run_bass_kernel_spmd(nc, in_maps, core_ids=list(range(M_CORES)))
    out = np.concatenate([res.results[i]["out"] for i in range(M_CORES)],
                         axis=0)
    return out.reshape(1, N_SLOTS, D).astype(np.float32)
